# revision 11
# baseline (speedup 1.0000x reference)
"""DeepSigNet Trainium2 kernel (8 NeuronCores, data-parallel over batch).

Restructured depth-3 streamed path-signature + 1x1-conv network:
all sequential scans become free-dim cumsums (tensor_tensor_scan), all
signature/projection contractions become TensorE matmuls in float32r (tf32),
rank-1 outer-product structures are built via 0/1 selection matmuls + one
elementwise multiply.

Math (per batch element, channels C, increments dx_t = a_t - a_{t-1}, a_{-1}=0):
  s1_t = cumsum(dx),  s1p = exclusive prefix,  u = s1p + dx/2
  s2[i,j]_t = cumsum_t(u[i] dx[j])
  y3_t = cumsum_v g3_v with
  g3_v[h] = sum_k dx_v[k] * sum_{ij} W3[h,i,j,k] (s2p_v[i,j] + dx_v[i]dx_v[j]/6)
          + sum_i s1p_v[i] * sum_{jk} W3[h,i,j,k] dx_v[j]dx_v[k]/2
Projection of each signature level commutes with the cumsum, so nothing of
size C^3 is ever materialized; per-step work is dense matmuls over L=256.

Self-contained: hardcodes shapes from the problem spec
(x: (8, 256, 33) f32; W1 (8,40494); b1 (8,); W2 (4,8); b2 (4,); Wl (1,84); bl (1,)).
"""
from contextlib import ExitStack

import numpy as np

import concourse.bass as bass
import concourse.tile as tile
from concourse import mybir
from concourse.bass_utils import run_bass_kernel_spmd

F32 = mybir.dt.float32
F32R = mybir.dt.float32r
AO = mybir.AluOpType
AF = mybir.ActivationFunctionType

B, L, CIN = 8, 256, 33
C = CIN + 1          # 34
H = 8                # conv1 out channels
C2 = 4               # conv2 out channels = stage-2 path channels
NPAIR = C * C        # 1156
NSYM = C * (C + 1) // 2   # 595
NP2 = C2 * C2        # 16
NSYM2 = C2 * (C2 + 1) // 2  # 10
KH = H * C           # 272 (cols (h,k) / (h,i), h-major)

# ---------------------------------------------------------------------------
# Walrus in this environment rejects >1 sync wait/update per instruction;
# split extras onto NOP carriers (a preceding same-engine NOP wait is
# semantically identical).
MAX_WAITS = 1
MAX_UPD = 1


def _fix_multiwait(nc):
    for func in nc.m.functions:
        for block in func.blocks:
            new_insts = []
            for inst in block.instructions:
                si = inst.sync_info
                if si is not None and si.on_wait and len(si.on_wait) > MAX_WAITS:
                    waits = list(si.on_wait)
                    for w in waits[MAX_WAITS:]:
                        new_insts.append(mybir.InstNoOp(
                            name=nc.get_next_instruction_name(), ins=[], outs=[],
                            engine=inst.engine,
                            sync_info=mybir.SyncInfo(on_wait=[w], on_update=[])))
                    inst.sync_info = mybir.SyncInfo(
                        on_wait=waits[:MAX_WAITS],
                        on_update=list(si.on_update or []))
                new_insts.append(inst)
                si = inst.sync_info
                if si is not None and si.on_update and len(si.on_update) > MAX_UPD:
                    assert not type(inst).__name__.startswith("InstDMA")
                    upds = list(si.on_update)
                    inst.sync_info = mybir.SyncInfo(
                        on_wait=list(si.on_wait or []), on_update=upds[:MAX_UPD])
                    for u in upds[MAX_UPD:]:
                        new_insts.append(mybir.InstNoOp(
                            name=nc.get_next_instruction_name(), ins=[], outs=[],
                            engine=inst.engine,
                            sync_info=mybir.SyncInfo(on_wait=[], on_update=[u])))
            block.instructions[:] = new_insts
    return nc


# ---------------------------------------------------------------------------
# host-side constant prep

def _sym_pairs(c):
    return [(p, q) for p in range(c) for q in range(p, c)]


def _chunks(n, size=128):
    return [(s, min(s + size, n)) for s in range(0, n, size)]


def _cycled_runs(lo, hi, period):
    """Dest rows [lo,hi) with src row = r % period -> contiguous runs
    (dest_lo, dest_hi, src_lo, src_hi)."""
    runs = []
    r = lo
    while r < hi:
        i = r % period
        n = min(period - i, hi - r)
        runs.append((r, r + n, i, i + n))
        r += n
    return runs


def _block_runs(pairs, lo, hi):
    """Dest rows [lo,hi) of the sym-pair table with src row = pairs[r][1]."""
    runs = []
    r = lo
    while r < hi:
        p, q = pairs[r]
        n = 1
        while r + n < hi and pairs[r + n] == (p, q + n):
            n += 1
        runs.append((r, r + n, q, q + n))
        r += n
    return runs


def prep_consts(W1, b1, W2, b2, Wl, bl):
    W1 = np.asarray(W1, np.float32)
    Wl = np.asarray(Wl, np.float32)
    W11 = W1[:, :C].T.copy()                                    # (34, 8)
    W12 = W1[:, C:C + NPAIR].reshape(H, C, C)                   # [h, i, j]
    W3 = W1[:, C + NPAIR:].reshape(H, C, C, C)                  # [h, i, j, k]
    W12p = W12.transpose(2, 1, 0).reshape(NPAIR, H).copy()      # [(j,i), h]
    W3A2 = W3.transpose(2, 1, 0, 3).reshape(NPAIR, KH).copy()   # [(j,i),(h,k)]
    pairs = _sym_pairs(C)
    W3hk = W3.transpose(1, 2, 0, 3).reshape(C, C, KH)           # [i, j, (h,k)]
    W3hi = W3.transpose(2, 3, 0, 1).reshape(C, C, KH)           # [j, k, (h,i)]
    W3S6 = np.zeros((NSYM, KH), np.float32)
    W3B2 = np.zeros((NSYM, KH), np.float32)
    for r, (p, q) in enumerate(pairs):
        if p == q:
            W3S6[r] = W3hk[p, p] / 6.0
            W3B2[r] = W3hi[p, p] / 2.0
        else:
            W3S6[r] = (W3hk[p, q] + W3hk[q, p]) / 6.0
            W3B2[r] = (W3hi[p, q] + W3hi[q, p]) / 2.0
    EJ = np.zeros((C, NPAIR), np.float32)
    for r in range(NPAIR):
        EJ[r // C, r] = 1.0
    EI = np.zeros((C, NSYM), np.float32)
    for r, (p, q) in enumerate(pairs):
        EI[p, r] = 1.0
    Ssel = np.zeros((KH, H), np.float32)
    for h in range(H):
        Ssel[h * C:(h + 1) * C, h] = 1.0

    Wl1 = Wl[:, :C2].T.copy()                                   # (4, 1)
    Wl2 = Wl[:, C2:C2 + NP2].reshape(C2, C2)                    # [i, j]
    Wl3 = Wl[:, C2 + NP2:].reshape(C2, C2, C2)                  # [i, j, k]
    Wl2p = Wl2.T.reshape(NP2, 1).copy()                         # [(j,i), 1]
    Wl3A2 = Wl3.transpose(1, 0, 2).reshape(NP2, C2).copy()      # [(j,i), k]
    pairs2 = _sym_pairs(C2)
    Wl3S6 = np.zeros((NSYM2, C2), np.float32)
    Wl3B2 = np.zeros((NSYM2, C2), np.float32)
    for r, (p, q) in enumerate(pairs2):
        if p == q:
            Wl3S6[r] = Wl3[p, p, :] / 6.0
            Wl3B2[r] = Wl3[:, p, p] / 2.0
        else:
            Wl3S6[r] = (Wl3[p, q, :] + Wl3[q, p, :]) / 6.0
            Wl3B2[r] = (Wl3[:, p, q] + Wl3[:, q, p]) / 2.0
    EJc = np.zeros((C2, NP2), np.float32)
    for r in range(NP2):
        EJc[r // C2, r] = 1.0
    EIc = np.zeros((C2, NSYM2), np.float32)
    for r, (p, q) in enumerate(pairs2):
        EIc[p, r] = 1.0

    return dict(
        W11=W11, W12p=W12p, W3A2=W3A2, W3S6=W3S6, W3B2=W3B2,
        EJ=EJ, EI=EI, Ssel=Ssel,
        W2T=np.asarray(W2, np.float32).T.copy(),
        b1=np.asarray(b1, np.float32).reshape(H, 1),
        b2c=np.asarray(b2, np.float32).reshape(C2, 1),
        bl=np.asarray(bl, np.float32).reshape(1, 1),
        Wl1=Wl1, Wl2p=Wl2p, Wl3A2=Wl3A2, Wl3S6=Wl3S6, Wl3B2=Wl3B2,
        EJc=EJc, EIc=EIc, onesc=np.ones((C2, 1), np.float32),
    )


TIME_ROW = np.linspace(0.0, 1.0, L, dtype=np.float32)[None, :]   # (1, 256)


# ---------------------------------------------------------------------------
# numpy mirror of the device dataflow (validation)

def np_forward(a_t, cst):
    pairs = _sym_pairs(C)
    inc = np.diff(a_t, axis=1, prepend=np.zeros((C, 1), np.float32))
    s1 = np.cumsum(inc, axis=1)
    s1p = np.concatenate([np.zeros((C, 1), np.float32), s1[:, :-1]], axis=1)
    u = inc * 0.5 + s1p
    dxj = cst["EJ"].T @ inc
    uj = u[np.arange(NPAIR) % C]
    pt = dxj * uj
    s2 = np.cumsum(pt, axis=1)
    s2p = np.concatenate([np.zeros((NPAIR, 1), np.float32), s2[:, :-1]], axis=1)
    dxi = cst["EI"].T @ inc
    dx2 = inc[[q for _, q in pairs]]
    b2t = dxi * dx2
    y12 = cst["W11"].T @ s1 + cst["W12p"].T @ s2
    M = cst["W3A2"].T @ s2p + cst["W3S6"].T @ b2t
    T = cst["W3B2"].T @ b2t
    dx3 = inc[np.arange(KH) % C]
    s1p3 = s1p[np.arange(KH) % C]
    g3 = cst["Ssel"].T @ (M * dx3) + cst["Ssel"].T @ (T * s1p3)
    y3 = np.cumsum(g3, axis=1)
    h = np.maximum(y12 + y3 + cst["b1"], 0.0)
    c = cst["W2T"].T @ h + cst["b2c"]
    pairs2 = _sym_pairs(C2)
    dc = np.diff(c, axis=1, prepend=np.zeros((C2, 1), np.float32))
    s1c = np.cumsum(dc, axis=1)
    s1cp = np.concatenate([np.zeros((C2, 1), np.float32), s1c[:, :-1]], axis=1)
    uc = dc * 0.5 + s1cp
    dcj = cst["EJc"].T @ dc
    uc4 = uc[np.arange(NP2) % C2]
    ptc = dcj * uc4
    s2c = np.cumsum(ptc, axis=1)
    s2cp = np.concatenate([np.zeros((NP2, 1), np.float32), s2c[:, :-1]], axis=1)
    dci = cst["EIc"].T @ dc
    dc2 = dc[[q for _, q in pairs2]]
    b2ct = dci * dc2
    yc = cst["Wl1"].T @ s1c + cst["Wl2p"].T @ s2c
    MC = cst["Wl3A2"].T @ s2cp + cst["Wl3S6"].T @ b2ct
    TC = cst["Wl3B2"].T @ b2ct
    g3c = cst["onesc"].T @ (MC * dc) + cst["onesc"].T @ (TC * s1cp)
    y3c = np.cumsum(g3c, axis=1)
    return yc + y3c + cst["bl"]


def np_kernel(x, W1, b1, W2, b2, Wl, bl):
    cst = prep_consts(W1, b1, W2, b2, Wl, bl)
    out = np.zeros((B, L, 1), np.float32)
    for b in range(B):
        a_t = np.concatenate([np.asarray(x[b], np.float32).T, TIME_ROW], 0)
        out[b, :, 0] = np_forward(a_t, cst)[0]
    return out


# ---------------------------------------------------------------------------
# bass program

def build_nc(use_f32r=True, reps=1, stop_at=None):
    DT = F32R if use_f32r else F32
    nc = bass.Bass()

    a_in = nc.dram_tensor("a_t", [C, L], F32, kind="ExternalInput")
    out_d = nc.dram_tensor("out", [1, L], F32, kind="ExternalOutput")

    def din(name, shape, dt=None):
        return nc.dram_tensor(name, list(shape), dt or DT,
                              kind="ExternalInput")

    d = {
        "W11": din("W11", (C, H)),
        "W12p": din("W12p", (NPAIR, H)),
        "W3A2": din("W3A2", (NPAIR, KH)),
        "W3S6": din("W3S6", (NSYM, KH)),
        "W3B2": din("W3B2", (NSYM, KH)),
        "EJ": din("EJ", (C, NPAIR)),
        "EI": din("EI", (C, NSYM)),
        "Ssel": din("Ssel", (KH, H)),
        "W2T": din("W2T", (H, C2)),
        "b1": din("b1", (H, 1), F32),
        "b2c": din("b2c", (C2, 1), F32),
        "bl": din("bl", (1, 1), F32),
        "Wl1": din("Wl1", (C2, 1)),
        "Wl2p": din("Wl2p", (NP2, 1)),
        "Wl3A2": din("Wl3A2", (NP2, C2)),
        "Wl3S6": din("Wl3S6", (NSYM2, C2)),
        "Wl3B2": din("Wl3B2", (NSYM2, C2)),
        "EJc": din("EJc", (C2, NP2)),
        "EIc": din("EIc", (C2, NSYM2)),
        "onesc": din("onesc", (C2, 1)),
    }

    pairs = _sym_pairs(C)
    pairs2 = _sym_pairs(C2)
    ch_np = _chunks(NPAIR)     # 10 chunks
    ch_ns = _chunks(NSYM)      # 5 chunks
    ch_kh = _chunks(KH)        # 3 chunks (128,128,16)

    with tile.TileContext(nc) as tc, ExitStack() as ctx:
        wpool = ctx.enter_context(tc.tile_pool(name="weights", bufs=1))
        spool = ctx.enter_context(tc.tile_pool(name="state", bufs=1))
        # PSUM budget (8 banks): selps 1 + mM 2 + mT 2 + small 1 + g3 1 = 7
        pse = ctx.enter_context(tc.tile_pool(name="psel", bufs=1, space="PSUM"))
        psm = ctx.enter_context(tc.tile_pool(name="psm", bufs=2, space="PSUM"))
        pst = ctx.enter_context(tc.tile_pool(name="pst", bufs=2, space="PSUM"))
        psy = ctx.enter_context(tc.tile_pool(name="psy", bufs=1, space="PSUM"))
        psg = ctx.enter_context(tc.tile_pool(name="psg", bufs=1, space="PSUM"))

        def wtile(name, shape, dt=DT, src=None):
            t = wpool.tile(list(shape), dt, name=name, tag=name)
            nc.sync.dma_start(t[:], src if src is not None else d[name][:])
            return t

        w11 = wtile("W11", (C, H))
        w12p = [wtile(f"W12p{g}", (hi - lo, H), src=d["W12p"][lo:hi, :])
                for g, (lo, hi) in enumerate(ch_np)]
        w3a2 = [wtile(f"W3A2{g}", (hi - lo, KH), src=d["W3A2"][lo:hi, :])
                for g, (lo, hi) in enumerate(ch_np)]
        w3s6 = [wtile(f"W3S6{g}", (hi - lo, KH), src=d["W3S6"][lo:hi, :])
                for g, (lo, hi) in enumerate(ch_ns)]
        w3b2 = [wtile(f"W3B2{g}", (hi - lo, KH), src=d["W3B2"][lo:hi, :])
                for g, (lo, hi) in enumerate(ch_ns)]
        ej = wtile("EJ", (C, NPAIR))
        ei = wtile("EI", (C, NSYM))
        ssel = [wtile(f"Ssel{g}", (hi - lo, H), src=d["Ssel"][lo:hi, :])
                for g, (lo, hi) in enumerate(ch_kh)]
        w2t = wtile("W2T", (H, C2))
        b1_sb = wtile("b1", (H, 1), F32)
        b2c_sb = wtile("b2c", (C2, 1), F32)
        bl_sb = wtile("bl", (1, 1), F32)
        wl1 = wtile("Wl1", (C2, 1))
        wl2p = wtile("Wl2p", (NP2, 1))
        wl3a2 = wtile("Wl3A2", (NP2, C2))
        wl3s6 = wtile("Wl3S6", (NSYM2, C2))
        wl3b2 = wtile("Wl3B2", (NSYM2, C2))
        ejc = wtile("EJc", (C2, NP2))
        eic = wtile("EIc", (C2, NSYM2))
        onesc = wtile("onesc", (C2, 1))

        zeros8 = wpool.tile([H, L], F32, name="zeros8", tag="zeros8")
        nc.gpsimd.memset(zeros8[:], 0.0)

        for rep in range(reps):
            def stile(name, shape, dt, bufs=1):
                return spool.tile(list(shape), dt, name=name, tag=name,
                                  bufs=max(bufs, 2 if reps > 1 else 1))

            def pstile(pool, name, shape):
                return pool.tile(list(shape), F32, name=name, tag=name)

            # ---- path load & increments -------------------------------
            abuf = stile("abuf", (C, L + 1), F32)
            nc.gpsimd.memset(abuf[:, 0:1], 0.0)
            nc.sync.dma_start(abuf[:, 1:L + 1], a_in[:])
            inc = stile("inc", (C, L), DT)
            nc.vector.tensor_sub(inc[:], abuf[:, 1:L + 1], abuf[:, 0:L])

            s1buf = stile("s1buf", (C, L + 1), DT)
            nc.gpsimd.memset(s1buf[:, 0:1].bitcast(F32), 0.0)
            nc.vector.tensor_tensor_scan(
                s1buf[:, 1:L + 1], inc[:], inc[:], 0.0,
                op0=AO.add, op1=AO.bypass)
            s1p = s1buf[:, 0:L]
            s1incl = s1buf[:, 1:L + 1]
            if stop_at == "s1":
                dbg = stile("dbg", (1, L), F32)
                nc.vector.tensor_copy(dbg[:], s1buf[0:1, 1:L + 1])
                nc.sync.dma_start(out_d[:], dbg[:])
                continue

            u_sb = stile("u_sb", (C, L), F32)
            nc.vector.scalar_tensor_tensor(
                u_sb[:], inc[:], 0.5, s1p, op0=AO.mult, op1=AO.add)

            # ---- P^T tiles and s2 scans -------------------------------
            s2bufs = []
            for g, (lo, hi) in enumerate(ch_np):
                n = hi - lo
                sel_ps = pse.tile([128, L], F32, name="selps", tag="selps",
                                  bufs=2)
                nc.tensor.matmul(sel_ps[0:n, :], ej[:, lo:hi], inc[:],
                                 start=True, stop=True)
                uj = stile(f"uj{g}", (n, L), F32)
                for (dlo, dhi, slo, shi) in _cycled_runs(lo, hi, C):
                    nc.sync.dma_start(uj[dlo - lo:dhi - lo, :],
                                      u_sb[slo:shi, :])
                pt = stile(f"pt{g}", (n, L), DT)
                nc.vector.tensor_mul(pt[:], sel_ps[0:n, :], uj[:])
                s2b = stile(f"s2buf{g}", (n, L + 1), DT)
                nc.gpsimd.memset(s2b[:, 0:1].bitcast(F32), 0.0)
                nc.vector.tensor_tensor_scan(
                    s2b[:, 1:L + 1], pt[:], pt[:], 0.0,
                    op0=AO.add, op1=AO.bypass)
                s2bufs.append(s2b)
                if stop_at == "s2_first" and g == 0:
                    break
            if stop_at in ("s2", "s2_first"):
                dbg = stile("dbg", (1, L), F32)
                nc.vector.tensor_copy(dbg[:], s2bufs[0][0:1, 1:L + 1])
                nc.sync.dma_start(out_d[:], dbg[:])
                continue

            # ---- B2T tiles -------------------------------------------
            b2ts = []
            for g, (lo, hi) in enumerate(ch_ns):
                n = hi - lo
                sel_ps = pse.tile([128, L], F32, name="selps", tag="selps",
                                  bufs=2)
                nc.tensor.matmul(sel_ps[0:n, :], ei[:, lo:hi], inc[:],
                                 start=True, stop=True)
                dx2 = stile(f"dx2{g}", (n, L), DT)
                for (dlo, dhi, slo, shi) in _block_runs(pairs, lo, hi):
                    nc.sync.dma_start(dx2[dlo - lo:dhi - lo, :],
                                      inc[slo:shi, :])
                b2t = stile(f"b2t{g}", (n, L), DT)
                nc.vector.tensor_mul(b2t[:], sel_ps[0:n, :], dx2[:])
                b2ts.append(b2t)
            if stop_at == "b2t":
                dbg = stile("dbg", (1, L), F32)
                nc.vector.tensor_copy(dbg[:], b2ts[0][0:1, 0:L])
                nc.sync.dma_start(out_d[:], dbg[:])
                continue

            # ---- y12 = W11.s1 + W12p.s2 ------------------------------
            y12_ps = pstile(psy, "ps_y12", (H, L))
            nc.tensor.matmul(y12_ps[:], w11[:], s1incl,
                             start=True, stop=False)
            for g, (lo, hi) in enumerate(ch_np):
                n = hi - lo
                nc.tensor.matmul(y12_ps[:], w12p[g][:],
                                 s2bufs[g][0:n, 1:L + 1],
                                 start=False, stop=(g == len(ch_np) - 1))

            if stop_at == "y12":
                tmp_y12 = stile("dbg_y12", (H, L), F32)
                nc.vector.tensor_copy(tmp_y12[:], y12_ps[:])
                nc.sync.dma_start(out_d[:], tmp_y12[0:1, :])
                continue

            # ---- per column-tile: M, T, Z, g3 accumulation -----------
            g3_ps = pstile(psg, "ps_g3", (H, L))
            for mt, (clo, chi) in enumerate(ch_kh):
                mn = chi - clo
                mp = psm.tile([mn, L], F32, name="ps_m", tag="ps_m", bufs=2)
                for g, (lo, hi) in enumerate(ch_np):
                    n = hi - lo
                    nc.tensor.matmul(mp[:], w3a2[g][:, clo:chi],
                                     s2bufs[g][0:n, 0:L],
                                     start=(g == 0), stop=False)
                for g, (lo, hi) in enumerate(ch_ns):
                    nc.tensor.matmul(mp[:], w3s6[g][:, clo:chi], b2ts[g][:],
                                     start=False, stop=(g == len(ch_ns) - 1))
                dx3 = stile(f"dx3_{mt}", (mn, L), DT)
                for (dlo, dhi, slo, shi) in _cycled_runs(clo, chi, C):
                    nc.sync.dma_start(dx3[dlo - clo:dhi - clo, :],
                                      inc[slo:shi, :])
                za = stile(f"za{mt}", (mn, L), DT)
                nc.vector.tensor_mul(za[:], mp[:], dx3[:])
                nc.tensor.matmul(g3_ps[:], ssel[mt][:], za[:],
                                 start=(mt == 0), stop=False)

                tp = pst.tile([mn, L], F32, name="ps_t", tag="ps_t", bufs=2)
                for g, (lo, hi) in enumerate(ch_ns):
                    nc.tensor.matmul(tp[:], w3b2[g][:, clo:chi], b2ts[g][:],
                                     start=(g == 0), stop=(g == len(ch_ns) - 1))
                s1p3 = stile(f"s1p3_{mt}", (mn, L), DT)
                for (dlo, dhi, slo, shi) in _cycled_runs(clo, chi, C):
                    nc.sync.dma_start(s1p3[dlo - clo:dhi - clo, :],
                                      s1buf[slo:shi, 0:L])
                zb = stile(f"zb{mt}", (mn, L), DT)
                nc.vector.tensor_mul(zb[:], tp[:], s1p3[:])
                nc.tensor.matmul(g3_ps[:], ssel[mt][:], zb[:],
                                 start=False, stop=(mt == len(ch_kh) - 1))

            if stop_at == "g3":
                tmp_g3 = stile("dbg_g3", (H, L), F32)
                nc.vector.tensor_copy(tmp_g3[:], g3_ps[:])
                nc.sync.dma_start(out_d[:], tmp_g3[0:1, :])
                continue
            y3_sb = stile("y3", (H, L), F32)
            nc.vector.tensor_tensor_scan(
                y3_sb[:], g3_ps[:], zeros8[:], 0.0,
                op0=AO.add, op1=AO.bypass)

            # ---- h = relu(y12 + y3 + b1); c = W2 h + b2 ---------------
            ypre = stile("ypre", (H, L), F32)
            nc.vector.tensor_add(ypre[:], y12_ps[:], y3_sb[:])
            hrelu = stile("hrelu", (H, L), DT)
            nc.scalar.activation(hrelu[:], ypre[:], AF.Relu, bias=b1_sb[:])
            c_ps = pstile(psy, "ps_y12", (C2, L))
            nc.tensor.matmul(c_ps[:], w2t[:], hrelu[:], start=True, stop=True)
            cbuf = stile("cbuf", (C2, L + 1), F32)
            nc.gpsimd.memset(cbuf[:, 0:1], 0.0)
            nc.scalar.activation(cbuf[:, 1:L + 1], c_ps[:], AF.Identity,
                                 bias=b2c_sb[:])

            if stop_at == "c":
                nc.sync.dma_start(out_d[:], cbuf[0:1, 1:L + 1])
                continue

            # ---- stage 2 ----------------------------------------------
            dc = stile("dc", (C2, L), DT)
            nc.vector.tensor_sub(dc[:], cbuf[:, 1:L + 1], cbuf[:, 0:L])
            s1cbuf = stile("s1cbuf", (C2, L + 1), DT)
            nc.gpsimd.memset(s1cbuf[:, 0:1].bitcast(F32), 0.0)
            nc.vector.tensor_tensor_scan(
                s1cbuf[:, 1:L + 1], dc[:], dc[:], 0.0,
                op0=AO.add, op1=AO.bypass)
            s1cp = s1cbuf[:, 0:L]
            uc = stile("uc", (C2, L), F32)
            nc.vector.scalar_tensor_tensor(
                uc[:], dc[:], 0.5, s1cp, op0=AO.mult, op1=AO.add)

            selc_ps = pse.tile([128, L], F32, name="selps", tag="selps",
                               bufs=2)
            nc.tensor.matmul(selc_ps[0:NP2, :], ejc[:], dc[:],
                             start=True, stop=True)
            uc4 = stile("uc4", (NP2, L), F32)
            for (dlo, dhi, slo, shi) in _cycled_runs(0, NP2, C2):
                nc.sync.dma_start(uc4[dlo:dhi, :], uc[slo:shi, :])
            ptc = stile("ptc", (NP2, L), DT)
            nc.vector.tensor_mul(ptc[:], selc_ps[0:NP2, :], uc4[:])
            s2cbuf = stile("s2cbuf", (NP2, L + 1), DT)
            nc.gpsimd.memset(s2cbuf[:, 0:1].bitcast(F32), 0.0)
            nc.vector.tensor_tensor_scan(
                s2cbuf[:, 1:L + 1], ptc[:], ptc[:], 0.0,
                op0=AO.add, op1=AO.bypass)

            seli_ps = pse.tile([128, L], F32, name="selps", tag="selps",
                               bufs=2)
            nc.tensor.matmul(seli_ps[0:NSYM2, :], eic[:], dc[:],
                             start=True, stop=True)
            dc2 = stile("dc2", (NSYM2, L), DT)
            for (dlo, dhi, slo, shi) in _block_runs(pairs2, 0, NSYM2):
                nc.sync.dma_start(dc2[dlo:dhi, :], dc[slo:shi, :])
            b2ct = stile("b2ct", (NSYM2, L), DT)
            nc.vector.tensor_mul(b2ct[:], seli_ps[0:NSYM2, :], dc2[:])

            if stop_at == "b2ct2":
                dbg2 = stile("dbg", (1, L), F32)
                nc.vector.tensor_copy(dbg2[:], b2ct[0:1, :])
                nc.sync.dma_start(out_d[:], dbg2[:])
                continue
            yc_ps = pstile(psy, "ps_y12", (1, L))
            nc.tensor.matmul(yc_ps[:], wl1[:], s1cbuf[:, 1:L + 1],
                             start=True, stop=False)
            nc.tensor.matmul(yc_ps[:], wl2p[:], s2cbuf[:, 1:L + 1],
                             start=False, stop=True)
            if stop_at == "yc":
                dbg3 = stile("dbg", (1, L), F32)
                nc.vector.tensor_copy(dbg3[:], yc_ps[:])
                nc.sync.dma_start(out_d[:], dbg3[:])
                continue
            mc_ps = psm.tile([C2, L], F32, name="ps_m", tag="ps_m", bufs=2)
            nc.tensor.matmul(mc_ps[:], wl3a2[:], s2cbuf[:, 0:L],
                             start=True, stop=False)
            nc.tensor.matmul(mc_ps[:], wl3s6[:], b2ct[:],
                             start=False, stop=True)
            tc_ps = pst.tile([C2, L], F32, name="ps_t", tag="ps_t", bufs=2)
            nc.tensor.matmul(tc_ps[:], wl3b2[:], b2ct[:],
                             start=True, stop=True)

            if stop_at == "mc":
                dbg4 = stile("dbg", (1, L), F32)
                nc.vector.tensor_copy(dbg4[:], mc_ps[0:1, :])
                nc.sync.dma_start(out_d[:], dbg4[:])
                continue
            zac = stile("zac", (C2, L), DT)
            nc.vector.tensor_mul(zac[:], mc_ps[:], dc[:])
            zbc = stile("zbc", (C2, L), DT)
            nc.vector.tensor_mul(zbc[:], tc_ps[:], s1cp)
            g3c_ps = pstile(psg, "ps_g3", (1, L))
            nc.tensor.matmul(g3c_ps[:], onesc[:], zac[:],
                             start=True, stop=False)
            nc.tensor.matmul(g3c_ps[:], onesc[:], zbc[:],
                             start=False, stop=True)
            if stop_at == "g3cc":
                dbg5 = stile("dbg", (1, L), F32)
                nc.vector.tensor_copy(dbg5[:], g3c_ps[:])
                nc.sync.dma_start(out_d[:], dbg5[:])
                continue
            y3c = stile("y3c", (1, L), F32)
            nc.vector.tensor_tensor_scan(
                y3c[:], g3c_ps[:], zeros8[0:1, :], 0.0,
                op0=AO.add, op1=AO.bypass)
            if stop_at == "y3cc":
                nc.sync.dma_start(out_d[:], y3c[:])
                continue
            osum = stile("osum", (1, L), F32)
            nc.vector.tensor_add(osum[:], y3c[:], yc_ps[:])
            out_sb = stile("out_sb", (1, L), F32)
            nc.scalar.activation(out_sb[:], osum[:], AF.Identity,
                                 bias=bl_sb[:])
            nc.sync.dma_start(out_d[:], out_sb[:])

    _fix_multiwait(nc)
    return nc


# ---------------------------------------------------------------------------
_CACHE = {}


def _get_nc(use_f32r=True, reps=1, stop_at=None):
    key = (use_f32r, reps, stop_at)
    if key not in _CACHE:
        _CACHE[key] = build_nc(use_f32r=use_f32r, reps=reps, stop_at=stop_at)
    return _CACHE[key]


def make_in_maps(x, W1, b1, W2, b2, Wl, bl):
    cst = prep_consts(W1, b1, W2, b2, Wl, bl)
    consts = {k: np.ascontiguousarray(v, np.float32) for k, v in cst.items()}
    in_maps = []
    for b in range(B):
        a_t = np.concatenate(
            [np.asarray(x[b], np.float32).T, TIME_ROW], axis=0)
        m = dict(consts)
        m["a_t"] = np.ascontiguousarray(a_t)
        in_maps.append(m)
    return in_maps, cst


def run(x, W1, b1, W2, b2, Wl, bl, use_f32r=True, reps=1, stop_at=None, **kwargs):
    nc = _get_nc(use_f32r=use_f32r, reps=reps, stop_at=stop_at)
    in_maps, _ = make_in_maps(x, W1, b1, W2, b2, Wl, bl)
    return run_bass_kernel_spmd(nc, in_maps, core_ids=list(range(B)), **kwargs)


def kernel(x, W1, b1, W2, b2, Wl, bl):
    res = run(x, W1, b1, W2, b2, Wl, bl)
    out = np.stack([res.results[b]["out"].reshape(L, 1) for b in range(B)])
    return out.astype(np.float32)
